# revision 1
# baseline (speedup 1.0000x reference)
"""PSANet COLLECT gather kernel for Trainium2 (8 NeuronCores).

out[0, oh*60+ow, h, w] = x[0, (oh+59-h)*119 + (ow+59-w), h, w]

Strategy: data-parallel over the 60 h-rows (8 rows per core, padded).
Per core the partition axis is the diagonal index i = oh+59-h (constant per
(oh, h) pair within a 4-row block), so the channel-gather becomes 60
same-partition free-axis-shifted copies (one per w), with all DMA transfers
made of contiguous >=240B runs.
"""

import numpy as np

H = 60
W = 60
R = 2 * H - 1          # 119
CIN = R * R            # 14161
HB = 8                 # padded h-rows per core
NI = 67                # i_loc values per shard: 60 + 8 - 1
N_CORES = 8

_COMPILED = {}


def _patch_tile_drain_and_legalize():
    """This walrus build allows at most ONE sync-wait per instruction.
    Patch TileContext's exit drain (which attaches one wait per tracked
    processor) and add a general pass splitting excess waits onto
    preceding same-engine NoOps."""
    import concourse.mybir as mybir
    from concourse.tile import TileContext
    from concourse.vector_clock import ScopedClock

    if getattr(TileContext, "_ant_drain_patched", False):
        return

    def _patched_drain_and_barrier(self, tick_clock, wait_clock):
        drain_inst = self.nc.sync.drain()
        wait_clock.add_sem_waits(
            drain_inst.ins, ScopedClock({None: tick_clock.global_clock})
        )
        si = drain_inst.ins.sync_info
        if si is not None and si.on_wait is not None and len(si.on_wait) > 1:
            waits = list(si.on_wait)
            drain_inst.ins.sync_info = mybir.SyncInfo(
                on_wait=waits[:1], on_update=list(si.on_update or [])
            )
            for i in range(1, len(waits)):
                nop = self.nc.sync.nop()
                nop.ins.sync_info = mybir.SyncInfo(on_wait=[waits[i]], on_update=[])
        self.nc.all_engine_barrier()
        assert self.sems is not None
        popped = self.nc._tile_sem_poison_stack.pop()
        assert popped is self._sem_poison
        self.nc.clear_and_free_semaphores(list(self.sems.allocated().values()))
        self.nc.all_engine_barrier()

    TileContext._drain_and_barrier = _patched_drain_and_barrier
    TileContext._ant_drain_patched = True


def _legalize_sync_waits(nc):
    """Split any instruction carrying >1 sync waits: hoist extras onto
    fresh same-engine NoOps inserted immediately before it."""
    import concourse.mybir as mybir

    counter = [0]
    for f in nc.m.functions:
        for bb in f.blocks:
            new_list = []
            for ins in bb.instructions:
                si = ins.sync_info
                if si is not None and si.on_wait is not None and len(si.on_wait) > 1:
                    waits = list(si.on_wait)
                    for wcmd in waits[:-1]:
                        nop = mybir.InstNoOp(
                            name=f"lgw-{counter[0]}", ins=[], outs=[], engine=ins.engine
                        )
                        counter[0] += 1
                        nop.sync_info = mybir.SyncInfo(on_wait=[wcmd], on_update=[])
                        new_list.append(nop)
                    ins.sync_info = mybir.SyncInfo(
                        on_wait=[waits[-1]], on_update=list(si.on_update or [])
                    )
                new_list.append(ins)
            bb.instructions = new_list


def _build_program():
    import concourse.bass as bass
    import concourse.mybir as mybir
    from concourse.tile import TileContext

    _patch_tile_drain_and_legalize()
    f32 = mybir.dt.float32

    nc = bass.Bass()
    xs = nc.declare_dram_parameter("xs", [NI, R, HB, W], f32, isOutput=False)
    out = nc.declare_dram_parameter("out", [H * W, HB, W], f32, isOutput=True)

    with TileContext(nc) as tc:
        with tc.tile_pool(name="p", bufs=1) as pool:
            Z = pool.tile([128, R * 4 * W], f32)    # per part: (j, hl, w)
            O = pool.tile([128, 4 * W * W], f32)    # per part: (hl, ow, w)

            Zv = Z[:, :].rearrange("p (j hl w) -> p j hl w", j=R, hl=4, w=W)
            # copy view: dims (p, ow<-j, hl, w)
            Ov_c = O[:, :].rearrange("p (hl ow w) -> p ow hl w", hl=4, ow=W, w=W)
            # store view: dims (p, hl, ow, w)
            Ov_s = O[:, :].rearrange("p (hl ow w) -> p hl ow w", hl=4, ow=W, w=W)

            out_v = out[:, :, :].rearrange("(oh ow) h w -> oh h ow w", oh=H, ow=W)

            # block A: partitions [0,63), i_loc = p+4,  h_loc = hl
            # block B: partitions [64,127), i_loc = p-64, h_loc = hl+4
            # loads (two HWDGE rings -> concurrent, complementary SBUF halves)
            nc.sync.dma_start(out=Zv[0:63], in_=xs[4:NI, :, 0:4, :])
            nc.scalar.dma_start(out=Zv[64:127], in_=xs[0:63, :, 4:8, :])

            # shear copies: O[p, hl, ow, w] = Z[p, ow+59-w, hl, w]
            for w in range(W):
                nc.vector.tensor_copy(
                    out=Ov_c[0:63, :, :, w : w + 1],
                    in_=Zv[0:63, (H - 1 - w) : (R - w), :, w : w + 1],
                )
                nc.gpsimd.tensor_copy(
                    out=Ov_c[64:127, :, :, w : w + 1],
                    in_=Zv[64:127, (H - 1 - w) : (R - w), :, w : w + 1],
                )

            # stores: oh = p_local - 3 + hl, valid p_local in [3-hl, 63-hl)
            for hl in range(4):
                nc.sync.dma_start(
                    out=out_v[:, hl, :, :],
                    in_=Ov_s[3 - hl : 63 - hl, hl, :, :],
                )
                nc.scalar.dma_start(
                    out=out_v[:, 4 + hl, :, :],
                    in_=Ov_s[64 + 3 - hl : 64 + 63 - hl, hl, :, :],
                )

    _legalize_sync_waits(nc)
    return nc


def _get_program():
    if "nc" not in _COMPILED:
        _COMPILED["nc"] = _build_program()
    return _COMPILED["nc"]


def kernel(x: np.ndarray) -> np.ndarray:
    from concourse.bass_utils import run_bass_kernel_spmd

    x = np.ascontiguousarray(x, dtype=np.float32)
    assert x.shape == (1, CIN, H, W), x.shape
    x4 = x.reshape(R, R, H, W)

    nc = _get_program()

    in_maps = []
    for k in range(N_CORES):
        sh = np.zeros((NI, R, HB, W), np.float32)
        i_lo = max(0, 8 * k - 52)
        hrows = min(HB, H - 8 * k)
        sh[i_lo:NI, :, 0:hrows, :] = x4[
            i_lo + 52 - 8 * k : 67 + 52 - 8 * k, :, 8 * k : 8 * k + hrows, :
        ]
        in_maps.append({"xs": sh})

    res = run_bass_kernel_spmd(nc, in_maps, list(range(N_CORES)))

    out = np.empty((1, H * W, H, W), np.float32)
    for k in range(N_CORES):
        hrows = min(HB, H - 8 * k)
        out[0, :, 8 * k : 8 * k + hrows, :] = res.results[k]["out"][:, 0:hrows, :]
    return out


# revision 2
# speedup vs baseline: 3.8177x; 3.8177x over previous
"""PSANet COLLECT gather kernel for Trainium2 (8 NeuronCores).

out[0, oh*60+ow, h, w] = x[0, (oh+59-h)*119 + (ow+59-w), h, w]

Strategy: data-parallel over the 60 h-rows (8 rows per core, padded).
Per core the partition axis is the diagonal index i = oh+59-h (constant per
(oh, h) pair within a 4-row block), so the channel-gather becomes 60
same-partition free-axis-shifted copies (one per w), with all DMA transfers
made of contiguous >=240B runs.
"""

import numpy as np

H = 60
W = 60
R = 2 * H - 1          # 119
CIN = R * R            # 14161
HB = 8                 # padded h-rows per core
NI = 67                # i_loc values per shard: 60 + 8 - 1
N_CORES = 8

_COMPILED = {}


def _patch_tile_drain_and_legalize():
    """This walrus build allows at most ONE sync-wait per instruction.
    Patch TileContext's exit drain (which attaches one wait per tracked
    processor) and add a general pass splitting excess waits onto
    preceding same-engine NoOps."""
    import concourse.mybir as mybir
    from concourse.tile import TileContext
    from concourse.vector_clock import ScopedClock

    if getattr(TileContext, "_ant_drain_patched", False):
        return

    def _patched_drain_and_barrier(self, tick_clock, wait_clock):
        drain_inst = self.nc.sync.drain()
        wait_clock.add_sem_waits(
            drain_inst.ins, ScopedClock({None: tick_clock.global_clock})
        )
        si = drain_inst.ins.sync_info
        if si is not None and si.on_wait is not None and len(si.on_wait) > 1:
            waits = list(si.on_wait)
            drain_inst.ins.sync_info = mybir.SyncInfo(
                on_wait=waits[:1], on_update=list(si.on_update or [])
            )
            for i in range(1, len(waits)):
                nop = self.nc.sync.nop()
                nop.ins.sync_info = mybir.SyncInfo(on_wait=[waits[i]], on_update=[])
        self.nc.all_engine_barrier()
        assert self.sems is not None
        popped = self.nc._tile_sem_poison_stack.pop()
        assert popped is self._sem_poison
        self.nc.clear_and_free_semaphores(list(self.sems.allocated().values()))
        self.nc.all_engine_barrier()

    TileContext._drain_and_barrier = _patched_drain_and_barrier
    TileContext._ant_drain_patched = True


def _legalize_sync_waits(nc):
    """Split any instruction carrying >1 sync waits: hoist extras onto
    fresh same-engine NoOps inserted immediately before it."""
    import concourse.mybir as mybir

    counter = [0]
    for f in nc.m.functions:
        for bb in f.blocks:
            new_list = []
            for ins in bb.instructions:
                si = ins.sync_info
                if si is not None and si.on_wait is not None and len(si.on_wait) > 1:
                    waits = list(si.on_wait)
                    for wcmd in waits[:-1]:
                        nop = mybir.InstNoOp(
                            name=f"lgw-{counter[0]}", ins=[], outs=[], engine=ins.engine
                        )
                        counter[0] += 1
                        nop.sync_info = mybir.SyncInfo(on_wait=[wcmd], on_update=[])
                        new_list.append(nop)
                    ins.sync_info = mybir.SyncInfo(
                        on_wait=[waits[-1]], on_update=list(si.on_update or [])
                    )
                new_list.append(ins)
            bb.instructions = new_list


def _build_program(reps: int = 1):
    import concourse.bass as bass
    import concourse.mybir as mybir
    from concourse.tile import TileContext

    _patch_tile_drain_and_legalize()
    f32 = mybir.dt.float32

    nc = bass.Bass()
    xs = nc.declare_dram_parameter("xs", [NI, R, HB, W], f32, isOutput=False)
    out = nc.declare_dram_parameter("out", [H * W, HB, W], f32, isOutput=True)

    with TileContext(nc) as tc:
        with tc.tile_pool(name="p", bufs=1) as pool:
            Z = pool.tile([128, R * 4 * W], f32)    # per part: (j, hl, w)
            O = pool.tile([128, 4 * W * W], f32)    # per part: (hl, ow, w)

            Zv = Z[:, :].rearrange("p (j hl w) -> p j hl w", j=R, hl=4, w=W)
            # copy view: dims (p, ow<-j, hl, w)
            Ov_c = O[:, :].rearrange("p (hl ow w) -> p ow hl w", hl=4, ow=W, w=W)
            # store view: dims (p, hl, ow, w)
            Ov_s = O[:, :].rearrange("p (hl ow w) -> p hl ow w", hl=4, ow=W, w=W)

            out_v = out[:, :, :].rearrange("(oh ow) h w -> oh h ow w", oh=H, ow=W)

            for _rep in range(reps):
                # block A: partitions [0,63), i_loc = p+4,  h_loc = hl
                # block B: partitions [64,127), i_loc = p-64, h_loc = hl+4
                # loads (two HWDGE rings -> concurrent, complementary halves)
                nc.sync.dma_start(out=Zv[0:63], in_=xs[4:NI, :, 0:4, :])
                nc.scalar.dma_start(out=Zv[64:127], in_=xs[0:63, :, 4:8, :])

                # shear copies: O[p, hl, ow, w] = Z[p, ow+59-w, hl, w]
                for w in range(W):
                    nc.vector.tensor_copy(
                        out=Ov_c[0:63, :, :, w : w + 1],
                        in_=Zv[0:63, (H - 1 - w) : (R - w), :, w : w + 1],
                    )
                    nc.gpsimd.tensor_copy(
                        out=Ov_c[64:127, :, :, w : w + 1],
                        in_=Zv[64:127, (H - 1 - w) : (R - w), :, w : w + 1],
                    )

                # stores: oh = p_local - 3 + hl, valid p_local in [3-hl, 63-hl)
                for hl in range(4):
                    nc.sync.dma_start(
                        out=out_v[:, hl, :, :],
                        in_=Ov_s[3 - hl : 63 - hl, hl, :, :],
                    )
                    nc.scalar.dma_start(
                        out=out_v[:, 4 + hl, :, :],
                        in_=Ov_s[64 + 3 - hl : 64 + 63 - hl, hl, :, :],
                    )

    _legalize_sync_waits(nc)
    return nc


def _get_program(reps: int = 1):
    if reps not in _COMPILED:
        _COMPILED[reps] = _build_program(reps)
    return _COMPILED[reps]


def kernel(x: np.ndarray) -> np.ndarray:
    from concourse.bass_utils import run_bass_kernel_spmd

    x = np.ascontiguousarray(x, dtype=np.float32)
    assert x.shape == (1, CIN, H, W), x.shape
    x4 = x.reshape(R, R, H, W)

    nc = _get_program()

    in_maps = []
    for k in range(N_CORES):
        sh = np.zeros((NI, R, HB, W), np.float32)
        i_lo = max(0, 8 * k - 52)
        hrows = min(HB, H - 8 * k)
        sh[i_lo:NI, :, 0:hrows, :] = x4[
            i_lo + 52 - 8 * k : 67 + 52 - 8 * k, :, 8 * k : 8 * k + hrows, :
        ]
        in_maps.append({"xs": sh})

    res = run_bass_kernel_spmd(nc, in_maps, list(range(N_CORES)))

    out = np.empty((1, H * W, H, W), np.float32)
    for k in range(N_CORES):
        hrows = min(HB, H - 8 * k)
        out[0, :, 8 * k : 8 * k + hrows, :] = res.results[k]["out"][:, 0:hrows, :]
    return out


# revision 5
# speedup vs baseline: 21.9713x; 5.7552x over previous
"""PSANet COLLECT gather kernel for Trainium2 (8 NeuronCores).

out[0, oh*60+ow, h, w] = x[0, (oh+59-h)*119 + (ow+59-w), h, w]

Strategy: data-parallel over the 60 h-rows (8 rows per core, padded).
Per core the partition axis is the diagonal index i = oh+59-h (constant per
(oh, h) pair within a 4-row block), so the channel-gather becomes 60
same-partition free-axis-shifted copies (one per w), with all DMA transfers
made of contiguous >=240B runs.
"""

import numpy as np

H = 60
W = 60
R = 2 * H - 1          # 119
CIN = R * R            # 14161
HB = 8                 # padded h-rows per core
NI = 67                # i_loc values per shard: 60 + 8 - 1
N_CORES = 8

_COMPILED = {}


def _patch_tile_drain_and_legalize():
    """This walrus build allows at most ONE sync-wait per instruction.
    Patch TileContext's exit drain (which attaches one wait per tracked
    processor) and add a general pass splitting excess waits onto
    preceding same-engine NoOps."""
    import concourse.mybir as mybir
    from concourse.tile import TileContext
    from concourse.vector_clock import ScopedClock

    if getattr(TileContext, "_ant_drain_patched", False):
        return

    def _patched_drain_and_barrier(self, tick_clock, wait_clock):
        drain_inst = self.nc.sync.drain()
        wait_clock.add_sem_waits(
            drain_inst.ins, ScopedClock({None: tick_clock.global_clock})
        )
        si = drain_inst.ins.sync_info
        if si is not None and si.on_wait is not None and len(si.on_wait) > 1:
            waits = list(si.on_wait)
            drain_inst.ins.sync_info = mybir.SyncInfo(
                on_wait=waits[:1], on_update=list(si.on_update or [])
            )
            for i in range(1, len(waits)):
                nop = self.nc.sync.nop()
                nop.ins.sync_info = mybir.SyncInfo(on_wait=[waits[i]], on_update=[])
        self.nc.all_engine_barrier()
        assert self.sems is not None
        popped = self.nc._tile_sem_poison_stack.pop()
        assert popped is self._sem_poison
        self.nc.clear_and_free_semaphores(list(self.sems.allocated().values()))
        self.nc.all_engine_barrier()

    TileContext._drain_and_barrier = _patched_drain_and_barrier
    TileContext._ant_drain_patched = True


def _legalize_sync_waits(nc):
    """Split any instruction carrying >1 sync waits: hoist extras onto
    fresh same-engine NoOps inserted immediately before it."""
    import concourse.mybir as mybir

    counter = [0]
    for f in nc.m.functions:
        for bb in f.blocks:
            new_list = []
            for ins in bb.instructions:
                si = ins.sync_info
                if si is not None and si.on_wait is not None and len(si.on_wait) > 1:
                    waits = list(si.on_wait)
                    for wcmd in waits[:-1]:
                        nop = mybir.InstNoOp(
                            name=f"lgw-{counter[0]}", ins=[], outs=[], engine=ins.engine
                        )
                        counter[0] += 1
                        nop.sync_info = mybir.SyncInfo(on_wait=[wcmd], on_update=[])
                        new_list.append(nop)
                    ins.sync_info = mybir.SyncInfo(
                        on_wait=[waits[-1]], on_update=list(si.on_update or [])
                    )
                new_list.append(ins)
            bb.instructions = new_list


def _build_program(reps: int = 1, variant: str = "all"):
    import concourse.bass as bass
    import concourse.mybir as mybir
    from concourse.tile import TileContext

    _patch_tile_drain_and_legalize()
    f32 = mybir.dt.float32

    nc = bass.Bass()
    xs = nc.declare_dram_parameter("xs", [NI, R, HB, W], f32, isOutput=False)
    out = nc.declare_dram_parameter("out", [H * W, HB, W], f32, isOutput=True)

    with TileContext(nc) as tc:
        with tc.tile_pool(name="p", bufs=1) as pool:
            Z = pool.tile([128, R * 4 * W], f32)    # per part: (j, hl, w)
            O = pool.tile([128, 4 * W * W], f32)    # per part: (hl, ow, w)

            Zv = Z[:, :].rearrange("p (j hl w) -> p j hl w", j=R, hl=4, w=W)
            # copy view: dims (p, ow<-j, hl, w)
            Ov_c = O[:, :].rearrange("p (hl ow w) -> p ow hl w", hl=4, ow=W, w=W)
            # store view: dims (p, hl, ow, w)
            Ov_s = O[:, :].rearrange("p (hl ow w) -> p hl ow w", hl=4, ow=W, w=W)

            out_v = out[:, :, :].rearrange("(oh ow) h w -> oh h ow w", oh=H, ow=W)

            if variant != "all":
                # variants leave one tile unwritten; give it a writer once
                nc.vector.memzero(Z[:, :])
                nc.vector.memzero(O[:, :])

            for _rep in range(reps):
                # block A: partitions [0,63), i_loc = p+4,  h_loc = hl
                # block B: partitions [64,127), i_loc = p-64, h_loc = hl+4
                # loads (two HWDGE rings -> concurrent, complementary halves)
                if variant in ("all", "dma"):
                    nc.sync.dma_start(out=Zv[0:63], in_=xs[4:NI, :, 0:4, :])
                    nc.scalar.dma_start(out=Zv[64:127], in_=xs[0:63, :, 4:8, :])

                # shear copies: O[p, hl, ow, w] = Z[p, ow+59-w, hl, w]
                if variant in ("all", "copy"):
                    for w in range(W):
                        nc.vector.tensor_copy(
                            out=Ov_c[0:63, :, :, w : w + 1],
                            in_=Zv[0:63, (H - 1 - w) : (R - w), :, w : w + 1],
                        )
                        nc.gpsimd.tensor_copy(
                            out=Ov_c[64:127, :, :, w : w + 1],
                            in_=Zv[64:127, (H - 1 - w) : (R - w), :, w : w + 1],
                        )

                # stores: oh = p_local - 3 + hl, valid p_local in [3-hl, 63-hl)
                if variant in ("all", "dma"):
                    for hl in range(4):
                        nc.sync.dma_start(
                            out=out_v[:, hl, :, :],
                            in_=Ov_s[3 - hl : 63 - hl, hl, :, :],
                        )
                        nc.scalar.dma_start(
                            out=out_v[:, 4 + hl, :, :],
                            in_=Ov_s[64 + 3 - hl : 64 + 63 - hl, hl, :, :],
                        )

    _legalize_sync_waits(nc)
    return nc


def _get_program(reps: int = 1, variant: str = "all"):
    key = (reps, variant)
    if key not in _COMPILED:
        _COMPILED[key] = _build_program(reps, variant)
    return _COMPILED[key]


def kernel(x: np.ndarray) -> np.ndarray:
    from concourse.bass_utils import run_bass_kernel_spmd

    x = np.ascontiguousarray(x, dtype=np.float32)
    assert x.shape == (1, CIN, H, W), x.shape
    x4 = x.reshape(R, R, H, W)

    nc = _get_program()

    in_maps = []
    for k in range(N_CORES):
        sh = np.zeros((NI, R, HB, W), np.float32)
        i_lo = max(0, 8 * k - 52)
        hrows = min(HB, H - 8 * k)
        sh[i_lo:NI, :, 0:hrows, :] = x4[
            i_lo + 52 - 8 * k : 67 + 52 - 8 * k, :, 8 * k : 8 * k + hrows, :
        ]
        in_maps.append({"xs": sh})

    res = run_bass_kernel_spmd(nc, in_maps, list(range(N_CORES)))

    out = np.empty((1, H * W, H, W), np.float32)
    for k in range(N_CORES):
        hrows = min(HB, H - 8 * k)
        out[0, :, 8 * k : 8 * k + hrows, :] = res.results[k]["out"][:, 0:hrows, :]
    return out
